# revision 26
# baseline (speedup 1.0000x reference)
"""AttentionRNNCell Trainium2 kernel.

Math (per batch row b):
  et[t]  = V_a . tanh( (h W_a + b_a) + x[t] U_a )        t in [0, TE)
  at     = exp(et);  s = sum(at)
  ctx    = (sum_t at[t] x[t]) / s
  zt     = sigmoid(h W_z + [inp, ctx] C_z + b_z)
  rt     = sigmoid(h W_r + [inp, ctx] C_r + b_r)
  tht    = tanh((rt*h) U_p + [inp, ctx] C_p + b_p)
  ht     = (1-zt)*h + zt*tht

Distribution: data-parallel over batch B=128 across 8 cores (16 rows each).
Everything not depending on x_seq (h W_*, inp C_*[:IN], biases) is folded on
the host into small per-core tensors; the device streams x once:
  - x cast-loaded fp32->fp8e4 (SWDGE cast DMA), natural [t, e] layout
  - ONE xbar transpose DMA per row-pair on the fp8 data viewed as bf16
    PAIRS, directly producing the fp8 DoubleRow rhs interleave
    (k = e = 2p+j) with the permutation folded into host-packed U_a
  - PE fp8 DoubleRow matmul (K=256 in one pass): uxpb = (16*U_a)^T x^T
  - ACT tanh (scale=1/16, per-partition bias wxpb) -> fp8 [u, t] tiles
  - PE V-dot: tanh tile stationary (fp8 FWL), 16*V_a streamed, N=1
  - ACT exp (scale=1/16, accum_out row sums) -> at fp8
  - PE context matmul (x fp8 natural stationary, at column streamed) -> ctx^T
  - gate matmuls in transposed [u, b] orientation; PE transpose -> ht
"""

from contextlib import ExitStack

import numpy as np
import ml_dtypes

import concourse.bass as bass
import concourse.mybir as mybir
import concourse.tile as tile

BF16 = ml_dtypes.bfloat16
FP8 = ml_dtypes.float8_e4m3
F32 = mybir.dt.float32
BF = mybir.dt.bfloat16
F8 = mybir.dt.float8e4
AF = mybir.ActivationFunctionType
DR = mybir.MatmulPerfMode.DoubleRow

B, TE, U, IN_DIM = 128, 2048, 256, 256
N_CORES = 8
BS = B // N_CORES  # 16 batch rows per core
P = 128
EC = U // P  # e-chunks (2)
UC = U // P  # u-chunks (2)
QSCALE = 16.0  # fp8 weight pre-scale (undone in ACT scale)


def split_multi_waits(nc, max_waits=1):
    """This container's walrus rejects instructions carrying more than one
    sync wait. Hoist extra waits onto standalone same-engine NoOps inserted
    immediately before the offending instruction (semantically identical:
    the engine blocks on each wait in order before executing it)."""
    n_new = 0
    for f in nc.m.functions:
        for blk in f.blocks:
            new_insts = []
            for inst in blk.instructions:
                si = inst.sync_info
                waits = list(si.on_wait) if si and si.on_wait else []
                if len(waits) > max_waits:
                    for w in waits[:-max_waits]:
                        nop = mybir.InstNoOp(
                            name=f"{inst.name}-hw{n_new}", ins=[], outs=[]
                        )
                        nop.engine = inst.engine
                        nop.sync_info = mybir.SyncInfo(on_wait=[w], on_update=[])
                        new_insts.append(nop)
                        n_new += 1
                    si.on_wait = waits[-max_waits:]
                new_insts.append(inst)
            blk.instructions = new_insts
    return n_new


def build_nc(bs=BS, te=TE, split_waits=True):
    """Build the per-core Bass module. Parametrized so a small variant can be
    simulated quickly; the production shape is (bs=16, te=2048)."""
    tc_n = te // P      # 128-col t-chunks (16)
    th_n = 2            # t halves
    t_half = te // th_n
    tq_n = t_half // P  # 128-col chunks per half
    n_mm = min(512, t_half)  # moving-operand chunk per DR matmul

    nc = bass.Bass()
    x_d = nc.declare_dram_parameter("x", [bs, te, U], F32, isOutput=False)
    ua8_d = nc.declare_dram_parameter("ua8", [P, EC, U], F8, isOutput=False)
    va8_d = nc.declare_dram_parameter("va8", [P, UC], F8, isOutput=False)
    wxpbT_d = nc.declare_dram_parameter("wxpbT", [U, bs], F32, isOutput=False)
    hT_d = nc.declare_dram_parameter("hT", [U, bs], F32, isOutput=False)
    g0T_d = nc.declare_dram_parameter("g0T", [3, U, bs], F32, isOutput=False)
    cz_d = nc.declare_dram_parameter("cz", [U, U], F32, isOutput=False)
    cr_d = nc.declare_dram_parameter("cr", [U, U], F32, isOutput=False)
    cp_d = nc.declare_dram_parameter("cp", [U, U], F32, isOutput=False)
    up_d = nc.declare_dram_parameter("up", [U, U], F32, isOutput=False)
    id_d = nc.declare_dram_parameter("ident", [P, P], F32, isOutput=False)
    ht_d = nc.declare_dram_parameter("ht", [bs, U], F32, isOutput=True)

    with tile.TileContext(nc) as tc, ExitStack() as ctx:
        singles = ctx.enter_context(tc.tile_pool(name="singles", bufs=1))
        x8_p = ctx.enter_context(tc.tile_pool(name="x8", bufs=8))
        xt_p = ctx.enter_context(tc.tile_pool(name="xt", bufs=8))
        vt8_p = ctx.enter_context(tc.tile_pool(name="vt8", bufs=4))
        at_p = ctx.enter_context(tc.tile_pool(name="at", bufs=4))
        small_p = ctx.enter_context(tc.tile_pool(name="small", bufs=4))
        ux_ps = ctx.enter_context(tc.tile_pool(name="uxps", bufs=2, space="PSUM"))
        et_ps = ctx.enter_context(tc.tile_pool(name="etps", bufs=2, space="PSUM"))
        ctx_ps = ctx.enter_context(tc.tile_pool(name="ctxps", bufs=2, space="PSUM"))

        # ---- setup: weights / small per-core tensors ----
        ua8_sb = singles.tile([P, EC, U], F8)
        nc.sync.dma_start(out=ua8_sb, in_=ua8_d[:, :, :])
        va8_sb = singles.tile([P, UC], F8)
        nc.sync.dma_start(out=va8_sb, in_=va8_d[:, :])
        wxpb_sb = singles.tile([P, UC, bs], F32)
        nc.sync.dma_start(out=wxpb_sb, in_=wxpbT_d[:, :].rearrange("(c p) b -> p c b", p=P))
        hT_sb = singles.tile([P, UC, bs], F32)
        nc.sync.dma_start(out=hT_sb, in_=hT_d[:, :].rearrange("(c p) b -> p c b", p=P))
        g0_sb = singles.tile([P, 3, UC, bs], F32)
        nc.sync.dma_start(out=g0_sb, in_=g0T_d[:, :, :].rearrange("g (c p) b -> p g c b", p=P))
        gate_w = {}
        for name, d in (("cz", cz_d), ("cr", cr_d), ("cp", cp_d), ("up", up_d)):
            w_sb = singles.tile([P, EC, U], F32, name=f"{name}_sb")
            nc.sync.dma_start(out=w_sb, in_=d[:, :].rearrange("(c p) u -> p c u", p=P))
            gate_w[name] = w_sb
        id_sb = singles.tile([P, P], F32)
        nc.sync.dma_start(out=id_sb, in_=id_d[:, :])
        ones_sb = singles.tile([P, P], F32)
        nc.vector.memset(ones_sb, 1.0)
        expsum_all = singles.tile([P, bs], F32)
        ctxT_all = singles.tile([P, EC, bs], F32)

        # ---- streaming loop over batch rows, loaded in pairs ----
        # One SWDGE cast-load + one xbar transpose per PAIR of rows halves
        # the DMA instruction count: the tile scheduler tracks DMA completion
        # on 8 shared semaphore lanes, and fewer DMAs per row widens the
        # effective in-flight window (lane recycling was serializing each
        # load behind the previous row's transpose).
        # x8[p, r, tc, e] = x[row, t = p*tc_n + tc, e] fp8 (cast during HBM
        # load; 2 contiguous 16KB reads per partition). The induced
        # t-permutation is consistent across the whole attention pipeline and
        # the final reductions over t are permutation-invariant.
        RP = 2  # rows per load
        x8_tiles, xt_tiles = {}, {}
        for b in range(bs):
            if b % RP == 0:
                nr = min(RP, bs - b)
                x8p = x8_p.tile([P, nr, tc_n, U], F8, name=f"x8_{b}", tag="x8p")
                nc.gpsimd.dma_start(
                    out=x8p,
                    in_=x_d[b : b + nr, :, :].rearrange("r (p tc) e -> p r tc e", p=P),
                )
                # One xbar transpose of bf16-viewed fp8 PAIRS:
                #   xt[p, r*tc_n + c, i] = pair (e=2p, e=2p+1) of row b+r at
                #   t = i*tc_n + c
                xtp = xt_p.tile([P, nr * tc_n, P], BF, name=f"xt_{b}", tag="xtp")
                nc.sync.dma_start(out=xtp, in_=x8p.bitcast(BF), transpose=True)
                for r in range(nr):
                    x8_tiles[b + r] = x8p[:, r, :, :]
                    xt_tiles[b + r] = xtp[:, r * tc_n : (r + 1) * tc_n, :]
            x8 = x8_tiles.pop(b)
            # fp8 view with DoubleRow rhs interleave: [p, j, t], k = e = 2p+j
            xt8 = xt_tiles.pop(b).bitcast(F8).rearrange("p c (i j) -> p j (c i)", j=2)

            vt8 = vt8_p.tile([P, UC, te], F8)  # tanh output [u%128, uc, t]
            et = et_ps.tile([P, tc_n], F32, tag="etps")
            for th in range(th_n):
                t0 = th * t_half
                for uc in range(UC):
                    ux = ux_ps.tile([P, t_half], F32, tag="ux", name=f"ux{uc}")
                    for n0 in range(0, t_half, n_mm):
                        nc.tensor.matmul(
                            out=ux[:, n0 : n0 + n_mm],
                            lhsT=ua8_sb[:, :, uc * P : (uc + 1) * P],
                            rhs=xt8[:, :, t0 + n0 : t0 + n0 + n_mm],
                            perf_mode=DR,
                            start=True,
                            stop=True,
                        )
                    nc.scalar.activation(
                        out=vt8[:, uc, t0 : t0 + t_half],
                        in_=ux,
                        func=AF.Tanh,
                        bias=wxpb_sb[:, uc, b : b + 1],
                        scale=1.0 / QSCALE,
                    )
                for tq in range(tq_n):
                    col = th * tq_n + tq
                    for uc in range(UC):
                        nc.tensor.matmul(
                            out=et[:, col : col + 1],
                            lhsT=vt8[:, uc, col * P : (col + 1) * P],
                            rhs=va8_sb[:, uc : uc + 1],
                            start=(uc == 0),
                            stop=(uc == UC - 1),
                        )

            at = at_p.tile([P, tc_n], F8)
            nc.scalar.activation(
                out=at, in_=et, func=AF.Exp, scale=1.0 / QSCALE,
                accum_out=expsum_all[:, b : b + 1],
            )

            # the two e-chunk context accumulators must live in different
            # PSUM banks (independent long-lived accumulation groups)
            cps = [
                ctx_ps.tile([P, 1], F32, tag="ctxps", name=f"cps{e}")
                for e in range(EC)
            ]
            for tcc in range(tc_n):
                for e in range(EC):
                    nc.tensor.matmul(
                        out=cps[e],
                        lhsT=x8[:, tcc, e * P : (e + 1) * P],
                        rhs=at[:, tcc : tcc + 1],
                        start=(tcc == 0),
                        stop=(tcc == tc_n - 1),
                    )
            for e in range(EC):
                nc.vector.tensor_copy(ctxT_all[:, e : e + 1, b], cps[e])

        # ---- tail: normalize context, gates, output ----
        s_ps = et_ps.tile([P, bs], F32, tag="etps", name="s_ps")
        nc.tensor.matmul(out=s_ps, lhsT=ones_sb, rhs=expsum_all)
        recips = small_p.tile([P, bs], F32)
        nc.vector.reciprocal(recips, s_ps)
        ctxn = singles.tile([P, EC, bs], F32)
        for e in range(EC):
            nc.vector.tensor_mul(ctxn[:, e, :], ctxT_all[:, e, :], recips)

        def gate_psum(w_names_rhs, name):
            """psum[uc] = sum over (w, rhs) pairs of w^T @ rhs, per u-chunk."""
            outs = []
            for uc in range(UC):
                g = et_ps.tile([P, bs], F32, tag="etps", name=f"{name}{uc}")
                n_mm2 = sum(EC for _ in w_names_rhs)
                i = 0
                for w_sb, rhs_fn in w_names_rhs:
                    for e in range(EC):
                        nc.tensor.matmul(
                            out=g,
                            lhsT=w_sb[:, e, uc * P : (uc + 1) * P],
                            rhs=rhs_fn(e),
                            start=(i == 0),
                            stop=(i == n_mm2 - 1),
                        )
                        i += 1
                outs.append(g)
            return outs

        # zt^T, rt^T = sigmoid(g0 + C_*ctx^T ctx^T)
        zt_sb = small_p.tile([P, UC, bs], F32)
        rt_sb = small_p.tile([P, UC, bs], F32)
        for gi, (wname, dst) in enumerate((("cz", zt_sb), ("cr", rt_sb))):
            gps = gate_psum([(gate_w[wname], lambda e: ctxn[:, e, :])], wname)
            for uc in range(UC):
                tmp = small_p.tile([P, bs], F32, tag="gtmp", name=f"t{wname}{uc}")
                nc.vector.tensor_add(tmp, gps[uc], g0_sb[:, gi, uc, :])
                nc.scalar.activation(out=dst[:, uc, :], in_=tmp, func=AF.Sigmoid)

        # rh^T = rt^T * h^T ; tht^T = tanh(g0p + U_p^T rh^T + C_pctx^T ctx^T)
        rh_sb = small_p.tile([P, UC, bs], F32)
        for uc in range(UC):
            nc.vector.tensor_mul(rh_sb[:, uc, :], rt_sb[:, uc, :], hT_sb[:, uc, :])
        gps = gate_psum(
            [(gate_w["up"], lambda e: rh_sb[:, e, :]), (gate_w["cp"], lambda e: ctxn[:, e, :])],
            "cp",
        )
        ht_nat = small_p.tile([bs, U], F32)
        for uc in range(UC):
            tmp = small_p.tile([P, bs], F32, tag="gtmp", name=f"tp{uc}")
            nc.vector.tensor_add(tmp, gps[uc], g0_sb[:, 2, uc, :])
            tht = small_p.tile([P, bs], F32, tag="gtmp", name=f"tht{uc}")
            nc.scalar.activation(out=tht, in_=tmp, func=AF.Tanh)
            # ht^T = h^T + zt^T*(tht^T - h^T)
            nc.vector.tensor_sub(tht, tht, hT_sb[:, uc, :])
            nc.vector.tensor_mul(tht, tht, zt_sb[:, uc, :])
            nc.vector.tensor_add(tht, tht, hT_sb[:, uc, :])
            tp = et_ps.tile([bs, P], F32, tag="etps", name=f"htp{uc}")
            nc.tensor.transpose(tp, tht, id_sb)
            nc.vector.tensor_copy(ht_nat[:, uc * P : (uc + 1) * P], tp)
        nc.sync.dma_start(out=ht_d[:, :], in_=ht_nat)

    if split_waits:
        split_multi_waits(nc)
    return nc


def _host_prep(inputs, h_tm, V_a, W_a, U_a, b_a, C_z, W_z, b_z, C_r, W_r, b_r,
               C_p, U_p, b_p):
    """Fold everything not depending on x_seq into small per-core tensors."""
    wxpb = h_tm @ W_a + b_a                                # [B, U]
    g_z0 = h_tm @ W_z + inputs @ C_z[:IN_DIM] + b_z        # [B, U]
    g_r0 = h_tm @ W_r + inputs @ C_r[:IN_DIM] + b_r
    g_p0 = inputs @ C_p[:IN_DIM] + b_p
    # fp8 packed attention weights (pre-scaled by QSCALE; ACT un-scales).
    # DoubleRow contraction index k = (partition p, subtile j) maps to
    # e = 2p + j — the pair-transpose interleave of x.
    ua8 = np.ascontiguousarray(
        (U_a.reshape(P, EC, U) * QSCALE).astype(FP8)
    )  # [P, EC, U]: ua8[p, j, u] = 16*U_a[2p+j, u]
    va8 = np.ascontiguousarray(
        (V_a.reshape(UC, P).T * QSCALE).astype(FP8)
    )  # [P, UC]: va8[p, j] = 16*V_a[j*128+p]
    shared = {
        "ua8": ua8,
        "va8": va8,
        "cz": np.ascontiguousarray(C_z[IN_DIM:].astype(np.float32)),
        "cr": np.ascontiguousarray(C_r[IN_DIM:].astype(np.float32)),
        "cp": np.ascontiguousarray(C_p[IN_DIM:].astype(np.float32)),
        "up": np.ascontiguousarray(U_p.astype(np.float32)),
        "ident": np.eye(P, dtype=np.float32),
    }
    per_core = []
    for c in range(N_CORES):
        s = slice(c * BS, (c + 1) * BS)
        per_core.append(
            {
                "wxpbT": np.ascontiguousarray(wxpb[s].T.astype(np.float32)),
                "hT": np.ascontiguousarray(h_tm[s].T.astype(np.float32)),
                "g0T": np.ascontiguousarray(
                    np.stack([g_z0[s].T, g_r0[s].T, g_p0[s].T]).astype(np.float32)
                ),
                **shared,
            }
        )
    return per_core




def kernel(inputs, h_tm, x_seq, V_a, W_a, U_a, b_a, C_z, W_z, b_z,
           C_r, W_r, b_r, C_p, U_p, b_p):
    from concourse.bass_utils import run_bass_kernel_spmd

    args = {k: np.asarray(v, dtype=np.float32) for k, v in dict(
        inputs=inputs, h_tm=h_tm, V_a=V_a, W_a=W_a, U_a=U_a, b_a=b_a,
        C_z=C_z, W_z=W_z, b_z=b_z, C_r=C_r, W_r=W_r, b_r=b_r,
        C_p=C_p, U_p=U_p, b_p=b_p).items()}
    x_seq = np.asarray(x_seq, dtype=np.float32)

    per_core = _host_prep(**args)
    in_maps = []
    for c in range(N_CORES):
        m = dict(per_core[c])
        m["x"] = np.ascontiguousarray(x_seq[c * BS : (c + 1) * BS])
        in_maps.append(m)

    nc = build_nc()
    res = run_bass_kernel_spmd(nc, in_maps, core_ids=list(range(N_CORES)))
    return np.concatenate([res.results[c]["ht"] for c in range(N_CORES)], axis=0)
